# revision 12
# baseline (speedup 1.0000x reference)
"""DotLoss kernel for Trainium2, data-parallel over 8 NeuronCores.

loss = mean_i[ relu(1 + dot(img[I[i]], aud[i]) - dot(img[i], aud[i]))
             + relu(1 + dot(img[i], aud[A[i]]) - dot(img[i], aud[i])) ]

Each core handles N/8 = 4096 rows: local rows stream in via contiguous
HWDGE DMAs (2MB per dma_start, 16KB contiguous per partition), impostor
rows via SWDGE dma_gather from the full (replicated) embedding tables in
device DRAM. Row dots are fused multiply+reduce (scalar_tensor_tensor) on
the vector engine. Each core emits a [128,1] partial hinge-sum; the host
sums partials and divides by N.

Row mapping: local chunk k holds rows k*LCHUNK + p*LSLOTS + c at
(partition p, slot c) — contiguous per partition for big DMA descriptors.
dma_gather position i lands at partition i%128, slot i//128, so the host
permutes each 512-row gather chunk's impostor indices to align with the
local layout. The summed loss is permutation-invariant; only the
per-row triple alignment matters.
"""

import numpy as np

N, D = 32768, 512
NCORES = 8
SHARD = N // NCORES          # 4096 rows per core
P = 128
LCHUNK = 1024                # rows per local-load chunk
NLCHUNK = SHARD // LCHUNK    # 4
LSLOTS = LCHUNK // P         # 8
GCHUNK = 512                 # rows per dma_gather
NGCHUNK = SHARD // GCHUNK    # 8
GSLOTS = GCHUNK // P         # 4
TSLOTS = SHARD // P          # 32 accumulator columns
IC = GCHUNK // 16            # idx columns per gather chunk (wrapped layout)

_CACHE = {}


def _build_nc():
    import concourse.bacc as bacc
    import concourse.mybir as mybir
    import concourse.tile as tile
    from contextlib import ExitStack

    fp32 = mybir.dt.float32
    i16 = mybir.dt.int16

    nc = bacc.Bacc("TRN2")
    img_full = nc.dram_tensor("img_full", [N, D], fp32, kind="ExternalInput")
    aud_full = nc.dram_tensor("aud_full", [N, D], fp32, kind="ExternalInput")
    img_loc = nc.dram_tensor("img_loc", [SHARD, D], fp32, kind="ExternalInput")
    aud_loc = nc.dram_tensor("aud_loc", [SHARD, D], fp32, kind="ExternalInput")
    iidx = nc.dram_tensor("iidx", [P, SHARD // 16], i16, kind="ExternalInput")
    aidx = nc.dram_tensor("aidx", [P, SHARD // 16], i16, kind="ExternalInput")
    partial = nc.dram_tensor("partial", [P, 1], fp32, kind="ExternalOutput")

    img_loc_r = img_loc.rearrange("(k p c) d -> k p (c d)", p=P, c=LSLOTS)
    aud_loc_r = aud_loc.rearrange("(k p c) d -> k p (c d)", p=P, c=LSLOTS)

    mult = mybir.AluOpType.mult
    add = mybir.AluOpType.add
    amax = mybir.AluOpType.max

    with ExitStack() as ctx:
        tc = ctx.enter_context(tile.TileContext(nc))
        lio = ctx.enter_context(tc.tile_pool(name="lio", bufs=2))
        gio = ctx.enter_context(tc.tile_pool(name="gio", bufs=4))
        idxp = ctx.enter_context(tc.tile_pool(name="idxp", bufs=1))
        acc = ctx.enter_context(tc.tile_pool(name="acc", bufs=1))
        scr = ctx.enter_context(tc.tile_pool(name="scr", bufs=6))

        # idx loads on SWDGE: keeps the gathers' prerequisite off the busy
        # HWDGE dispatch queue (Pool-local dependency chain).
        iidx_sb = idxp.tile([P, SHARD // 16], i16, tag="iidx")
        nc.gpsimd.dma_start(out=iidx_sb[:], in_=iidx[:])
        aidx_sb = idxp.tile([P, SHARD // 16], i16, tag="aidx")
        nc.gpsimd.dma_start(out=aidx_sb[:], in_=aidx[:])

        anchor = acc.tile([P, TSLOTS], fp32, tag="anchor")
        iimp = acc.tile([P, TSLOTS], fp32, tag="iimp")
        aimp = acc.tile([P, TSLOTS], fp32, tag="aimp")

        def dot(dst_col, a, b):
            pr = scr.tile([P, D], fp32, tag="pr")
            nc.vector.scalar_tensor_tensor(
                out=pr[:], in0=a, scalar=1.0, in1=b,
                op0=mult, op1=mult, accum_out=dst_col,
            )

        for k in range(NLCHUNK):
            # 512-row gathers, two per local chunk (emitted first: the Pool
            # descriptor-generation chain is the critical resource).
            gts = []
            for h in range(2):
                g = 2 * k + h
                gi = gio.tile([P, GSLOTS, D], fp32, tag="gi")
                nc.gpsimd.dma_gather(
                    out_ap=gi[:], in_ap=img_full[:],
                    idxs_ap=iidx_sb[:, g * IC:(g + 1) * IC],
                    num_idxs=GCHUNK, num_idxs_reg=GCHUNK, elem_size=D,
                )
                ga = gio.tile([P, GSLOTS, D], fp32, tag="ga")
                nc.gpsimd.dma_gather(
                    out_ap=ga[:], in_ap=aud_full[:],
                    idxs_ap=aidx_sb[:, g * IC:(g + 1) * IC],
                    num_idxs=GCHUNK, num_idxs_reg=GCHUNK, elem_size=D,
                )
                gts.append((gi, ga))

            li = lio.tile([P, LSLOTS, D], fp32, tag="li")
            nc.sync.dma_start(out=li[:].rearrange("p c d -> p (c d)"),
                              in_=img_loc_r[k])
            la = lio.tile([P, LSLOTS, D], fp32, tag="la")
            nc.sync.dma_start(out=la[:].rearrange("p c d -> p (c d)"),
                              in_=aud_loc_r[k])

            # anchors first: they only need the local chunk, so the DVE has
            # work while the gathers drain.
            for c in range(LSLOTS):
                dot(anchor[:, k * LSLOTS + c:k * LSLOTS + c + 1],
                    li[:, c], la[:, c])
            for h in range(2):
                gi, ga = gts[h]
                for c in range(GSLOTS):
                    col = k * LSLOTS + h * GSLOTS + c
                    dot(iimp[:, col:col + 1], gi[:, c], la[:, h * GSLOTS + c])
                for c in range(GSLOTS):
                    col = k * LSLOTS + h * GSLOTS + c
                    dot(aimp[:, col:col + 1], li[:, h * GSLOTS + c], ga[:, c])

        diff = acc.tile([P, 2 * TSLOTS], fp32, tag="diff")
        nc.vector.tensor_sub(diff[:, 0:TSLOTS], iimp[:], anchor[:])
        nc.vector.tensor_sub(diff[:, TSLOTS:], aimp[:], anchor[:])
        hout = acc.tile([P, 2 * TSLOTS], fp32, tag="hout")
        nc.vector.tensor_scalar(
            out=hout[:], in0=diff[:], scalar1=1.0, scalar2=0.0,
            op0=add, op1=amax,
        )
        psum_t = acc.tile([P, 1], fp32, tag="psum")
        nc.vector.tensor_reduce(
            out=psum_t[:], in_=hout[:], axis=mybir.AxisListType.X, op=add,
        )
        nc.sync.dma_start(out=partial[:], in_=psum_t[:])

    nc.compile()
    return nc


def _get_nc():
    if "nc" not in _CACHE:
        _CACHE["nc"] = _build_nc()
    return _CACHE["nc"]


def _prep_idx(imp_core):
    """Wrap one core's impostor indices into the dma_gather SBUF layout.

    Local row j = k*LCHUNK + p*LSLOTS + c maps to gather chunk
    g = 2k + c//GSLOTS at position i = (c % GSLOTS)*128 + p. The wrapped
    tile stores gather position i of chunk g at [i % 16, g*IC + i // 16],
    replicated across the 8 GPSIMD partition groups.
    """
    g4 = imp_core.reshape(NLCHUNK, P, 2, GSLOTS)           # [k, p, h, c']
    gi = np.transpose(g4, (0, 2, 3, 1)).reshape(NGCHUNK, GCHUNK)  # [g, c'*P+p]
    w = gi.reshape(NGCHUNK, IC, 16)
    w = np.transpose(w, (2, 0, 1)).reshape(16, SHARD // 16)  # [q, (g s)]
    return np.ascontiguousarray(np.tile(w, (8, 1)).astype(np.int16))


def make_in_maps(image_outputs, audio_outputs, I_imp_ind, A_imp_ind):
    img = np.ascontiguousarray(image_outputs, dtype=np.float32)
    aud = np.ascontiguousarray(audio_outputs, dtype=np.float32)
    I_imp = np.asarray(I_imp_ind).astype(np.int64)
    A_imp = np.asarray(A_imp_ind).astype(np.int64)
    in_maps = []
    for c in range(NCORES):
        base = c * SHARD
        in_maps.append({
            "img_full": img,
            "aud_full": aud,
            "img_loc": np.ascontiguousarray(img[base:base + SHARD]),
            "aud_loc": np.ascontiguousarray(aud[base:base + SHARD]),
            "iidx": _prep_idx(I_imp[base:base + SHARD]),
            "aidx": _prep_idx(A_imp[base:base + SHARD]),
        })
    return in_maps


def kernel(image_outputs, audio_outputs, I_imp_ind, A_imp_ind):
    from concourse import bass_utils

    nc = _get_nc()
    in_maps = make_in_maps(image_outputs, audio_outputs, I_imp_ind, A_imp_ind)
    res = bass_utils.run_bass_kernel_spmd(nc, in_maps, list(range(NCORES))).results
    total = sum(float(r["partial"].sum(dtype=np.float64)) for r in res)
    return np.float32(total / N)


# revision 13
# speedup vs baseline: 1.0475x; 1.0475x over previous
"""DotLoss kernel for Trainium2, data-parallel over 8 NeuronCores.

loss = mean_i[ relu(1 + dot(img[I[i]], aud[i]) - dot(img[i], aud[i]))
             + relu(1 + dot(img[i], aud[A[i]]) - dot(img[i], aud[i])) ]

Each core handles N/8 = 4096 rows: local rows stream in via contiguous
HWDGE DMAs (2MB per dma_start, 16KB contiguous per partition), impostor
rows via SWDGE dma_gather (1024 rows per call) from the full (replicated)
embedding tables in device DRAM. Row dots are fused multiply+reduce
(scalar_tensor_tensor) on the vector engine. Each core emits a [128,1]
partial hinge-sum; the host sums partials and divides by N.

Row mapping: chunk k holds rows k*CHUNK + p*SLOTS + c at (partition p,
slot c) — contiguous per partition for big DMA descriptors. dma_gather
position i lands at partition i%128, slot i//128, so the host permutes
each chunk's impostor indices with i = c*128 + p. The summed loss is
permutation-invariant; only the per-row triple alignment matters.
"""

import numpy as np

N, D = 32768, 512
NCORES = 8
SHARD = N // NCORES          # 4096 rows per core
P = 128
CHUNK = 1024                 # rows per chunk (local load + gather)
NCHUNK = SHARD // CHUNK      # 4
SLOTS = CHUNK // P           # 8
TSLOTS = SHARD // P          # 32 accumulator columns
IC = CHUNK // 16             # idx columns per chunk in the wrapped layout

_CACHE = {}


def _build_nc():
    import concourse.bacc as bacc
    import concourse.mybir as mybir
    import concourse.tile as tile
    from concourse import library_config
    from contextlib import ExitStack

    fp32 = mybir.dt.float32
    i16 = mybir.dt.int16

    nc = bacc.Bacc("TRN2")
    img_full = nc.dram_tensor("img_full", [N, D], fp32, kind="ExternalInput")
    aud_full = nc.dram_tensor("aud_full", [N, D], fp32, kind="ExternalInput")
    img_loc = nc.dram_tensor("img_loc", [SHARD, D], fp32, kind="ExternalInput")
    aud_loc = nc.dram_tensor("aud_loc", [SHARD, D], fp32, kind="ExternalInput")
    iidx = nc.dram_tensor("iidx", [P, SHARD // 16], i16, kind="ExternalInput")
    aidx = nc.dram_tensor("aidx", [P, SHARD // 16], i16, kind="ExternalInput")
    partial = nc.dram_tensor("partial", [P, 1], fp32, kind="ExternalOutput")

    img_loc_r = img_loc.rearrange("(k p c) d -> k p (c d)", p=P, c=SLOTS)
    aud_loc_r = aud_loc.rearrange("(k p c) d -> k p (c d)", p=P, c=SLOTS)

    mult = mybir.AluOpType.mult
    add = mybir.AluOpType.add
    amax = mybir.AluOpType.max

    with ExitStack() as ctx:
        tc = ctx.enter_context(tile.TileContext(nc))
        lio = ctx.enter_context(tc.tile_pool(name="lio", bufs=2))
        gio = ctx.enter_context(tc.tile_pool(name="gio", bufs=3))
        idxp = ctx.enter_context(tc.tile_pool(name="idxp", bufs=1))
        acc = ctx.enter_context(tc.tile_pool(name="acc", bufs=1))
        scr = ctx.enter_context(tc.tile_pool(name="scr", bufs=6))

        # Load the mlp GPSIMD library first: the Q7 ucode fetch takes ~15us
        # and gates the first dma_gather, so start it as early as possible.
        nc.gpsimd.load_library(library_config.mlp)

        iidx_sb = idxp.tile([P, SHARD // 16], i16, tag="iidx")
        nc.sync.dma_start(out=iidx_sb[:], in_=iidx[:])
        aidx_sb = idxp.tile([P, SHARD // 16], i16, tag="aidx")
        nc.sync.dma_start(out=aidx_sb[:], in_=aidx[:])

        anchor = acc.tile([P, TSLOTS], fp32, tag="anchor")
        iimp = acc.tile([P, TSLOTS], fp32, tag="iimp")
        aimp = acc.tile([P, TSLOTS], fp32, tag="aimp")

        def dot(dst_col, a, b):
            pr = scr.tile([P, D], fp32, tag="pr")
            nc.vector.scalar_tensor_tensor(
                out=pr[:], in0=a, scalar=1.0, in1=b,
                op0=mult, op1=mult, accum_out=dst_col,
            )

        for k in range(NCHUNK):
            gi = gio.tile([P, SLOTS, D], fp32, tag="gi")
            nc.gpsimd.dma_gather(
                out_ap=gi[:], in_ap=img_full[:],
                idxs_ap=iidx_sb[:, k * IC:(k + 1) * IC],
                num_idxs=CHUNK, num_idxs_reg=CHUNK, elem_size=D,
            )
            ga = gio.tile([P, SLOTS, D], fp32, tag="ga")
            nc.gpsimd.dma_gather(
                out_ap=ga[:], in_ap=aud_full[:],
                idxs_ap=aidx_sb[:, k * IC:(k + 1) * IC],
                num_idxs=CHUNK, num_idxs_reg=CHUNK, elem_size=D,
            )
            li = lio.tile([P, SLOTS, D], fp32, tag="li")
            nc.sync.dma_start(out=li[:].rearrange("p c d -> p (c d)"),
                              in_=img_loc_r[k])
            la = lio.tile([P, SLOTS, D], fp32, tag="la")
            nc.sync.dma_start(out=la[:].rearrange("p c d -> p (c d)"),
                              in_=aud_loc_r[k])

            # anchors first: they only need the local chunk, so the DVE has
            # work while this chunk's gathers drain.
            for c in range(SLOTS):
                col = k * SLOTS + c
                dot(anchor[:, col:col + 1], li[:, c], la[:, c])
            for c in range(SLOTS):
                col = k * SLOTS + c
                dot(iimp[:, col:col + 1], gi[:, c], la[:, c])
            for c in range(SLOTS):
                col = k * SLOTS + c
                dot(aimp[:, col:col + 1], li[:, c], ga[:, c])

        diff = acc.tile([P, 2 * TSLOTS], fp32, tag="diff")
        nc.vector.tensor_sub(diff[:, 0:TSLOTS], iimp[:], anchor[:])
        nc.vector.tensor_sub(diff[:, TSLOTS:], aimp[:], anchor[:])
        hout = acc.tile([P, 2 * TSLOTS], fp32, tag="hout")
        nc.vector.tensor_scalar(
            out=hout[:], in0=diff[:], scalar1=1.0, scalar2=0.0,
            op0=add, op1=amax,
        )
        psum_t = acc.tile([P, 1], fp32, tag="psum")
        nc.vector.tensor_reduce(
            out=psum_t[:], in_=hout[:], axis=mybir.AxisListType.X, op=add,
        )
        nc.sync.dma_start(out=partial[:], in_=psum_t[:])

    nc.compile()
    return nc


def _get_nc():
    if "nc" not in _CACHE:
        _CACHE["nc"] = _build_nc()
    return _CACHE["nc"]


def _prep_idx(imp_core):
    """Wrap one core's impostor indices into the dma_gather SBUF layout.

    Local row j = k*CHUNK + p*SLOTS + c is gathered by chunk k at position
    i = c*128 + p. The wrapped tile stores position i of chunk k at
    [i % 16, k*IC + i // 16], replicated across the 8 GPSIMD partition
    groups.
    """
    g = imp_core.reshape(NCHUNK, P, SLOTS)
    gi = np.transpose(g, (0, 2, 1)).reshape(NCHUNK, CHUNK)   # [k, c*P + p]
    w = gi.reshape(NCHUNK, IC, 16)
    w = np.transpose(w, (2, 0, 1)).reshape(16, SHARD // 16)  # [q, (k s)]
    return np.ascontiguousarray(np.tile(w, (8, 1)).astype(np.int16))


def make_in_maps(image_outputs, audio_outputs, I_imp_ind, A_imp_ind):
    img = np.ascontiguousarray(image_outputs, dtype=np.float32)
    aud = np.ascontiguousarray(audio_outputs, dtype=np.float32)
    I_imp = np.asarray(I_imp_ind).astype(np.int64)
    A_imp = np.asarray(A_imp_ind).astype(np.int64)
    in_maps = []
    for c in range(NCORES):
        base = c * SHARD
        in_maps.append({
            "img_full": img,
            "aud_full": aud,
            "img_loc": np.ascontiguousarray(img[base:base + SHARD]),
            "aud_loc": np.ascontiguousarray(aud[base:base + SHARD]),
            "iidx": _prep_idx(I_imp[base:base + SHARD]),
            "aidx": _prep_idx(A_imp[base:base + SHARD]),
        })
    return in_maps


def kernel(image_outputs, audio_outputs, I_imp_ind, A_imp_ind):
    from concourse import bass_utils

    nc = _get_nc()
    in_maps = make_in_maps(image_outputs, audio_outputs, I_imp_ind, A_imp_ind)
    res = bass_utils.run_bass_kernel_spmd(nc, in_maps, list(range(NCORES))).results
    total = sum(float(r["partial"].sum(dtype=np.float64)) for r in res)
    return np.float32(total / N)
